# revision 32
# baseline (speedup 1.0000x reference)
"""DSAttention layer for Trainium2, 8 NeuronCores.

Sharding: core c -> batch b = c//2, head-group g = c%2 (4 heads each,
e-columns 256g..256g+255 of the 512-wide head dim).  tau[b]/8 (softmax
temperature x 1/sqrt(E)) is folded into each core's Wq/bq slice on the
host; delta[b] broadcasts over the softmax axis and is shift-invariant,
so it drops out exactly.  Each core emits its head-group's partial
output projection [2048, 512] fp16; the host sums the pair per batch
and adds (bv @ Wo + bo) in fp32.

v3: ACT-engine (exp) is the pacer (~142us of exp work).  The kernel is
one continuous scores->exp->AV stream per (l-quarter, head-pair); all
projection work (k/v/q proj, output proj) is emitted interleaved into
the stream so the out-of-order Tile scheduler uses it as PE filler and
the ACT engine never starves.  Z-normalization without PE transposes:
DVE reciprocal directly on the PSUM Z rows ([1,512] costs the same as
[128,512] on DVE), then a K=1 ones matmul broadcasts 1/Z across 64
partitions.  attnT is stored head-pair-stacked [128, 2, L] (odd head
on partitions 64:127 via DVE quadrant-routed writes) so the output
projection runs K=128 full-height (2 matmuls per l-tile instead of 4).
Input DMAs are issued chunk-interleaved (k0 q0 v0 k1 v1 ... q1 q2 q3)
so the first scores start ~10us in.

Device dataflow per core (all matmul operands fp16, fp32 PSUM accum):
  xT[q|k|v] [128, 4, 2048] fp16 DMA'd directly (host pre-transposed)
  qT/kT [e 256, l 2048] = W^T @ X^T   (e on partitions)
  v     [s 2048, e 256] -> fp16 v_aug [s,65] per head (ones col -> Z)
  scoresT[s,l] = kT.T @ qT  per head, head pairs concurrent via
                 partition-offset row groups (K=64 at rows 0-63/64-127)
  E = exp(scoresT - 2) fp16  (one ACT instr per [128, 2x512] pair tile)
  attnT_aug[65,l] = v_aug.T @ E  (accumulate 16 s-chunks in PSUM;
                 row 64 = softmax denominator Z)
  normalize: 1/Z row (DVE recip, psum->sbuf) -> K=1 matmul broadcast
                 -> attnT[128, pair, l] fp16 in SBUF
  out[l,512] = sum_p attnT[:, p, lt].T @ Wo_pair[:, p, :]  (K=128 x2)
"""

import numpy as np
from contextlib import ExitStack

import concourse.bass as bass
import concourse.bacc as bacc
import concourse.mybir as mybir
import concourse.tile as tile
from concourse.bass_utils import run_bass_kernel_spmd

F32 = mybir.dt.float32
F16 = mybir.dt.float16

B, L, S, D = 4, 2048, 2048, 512
H, E = 8, 64          # full model heads / head dim
HG = 4                # heads per core (head-group)
EG = HG * E           # 256, e-columns per core
N_CORES = 8

LT = L // 128         # 16 l-tiles
ST = S // 128         # 16 s-tiles
DC = D // 128         # 4 d-chunks
LQ = 4                # l-quarters of 512
SCALE = 1.0 / np.sqrt(np.float32(E))
EXP_SHIFT = -2.0      # exp(x-2): cancels in softmax, guards fp16 overflow


def _emit(ctx: ExitStack, tc: "tile.TileContext", io: dict):
    nc = tc.nc
    mm = nc.tensor.matmul

    singles = ctx.enter_context(tc.tile_pool(name="singles", bufs=1))
    bigs = ctx.enter_context(tc.tile_pool(name="bigs", bufs=1))
    e_pool = ctx.enter_context(tc.tile_pool(name="eslab", bufs=8))
    z_pool = ctx.enter_context(tc.tile_pool(name="zrec", bufs=2))
    ob_pool = ctx.enter_context(tc.tile_pool(name="outsb", bufs=3))

    # PSUM, statically 8 banks: sc 2x2 + av 3x1 + work 1x1.
    ps = ctx.enter_context(tc.tile_pool(name="ps", bufs=2, space="PSUM"))
    ps_av = ctx.enter_context(tc.tile_pool(name="ps_av", bufs=3, space="PSUM"))
    ps_wk = ctx.enter_context(tc.tile_pool(name="ps_wk", bufs=1, space="PSUM"))

    # ---- constants & weights -------------------------------------------
    wq_sb = singles.tile([128, DC, EG], F16)   # [p, c, e] = Wq[c*128+p, e]
    wk_sb = singles.tile([128, DC, EG], F16)
    wv_sb = singles.tile([128, DC, EG], F16)
    wo_sb = singles.tile([128, 2, D], F16)     # [r, p, n] = Wo[128p+r, n]
    bq_sb = singles.tile([128, 2], F32)        # [p, ec] = bq[128ec+p]
    bk_sb = singles.tile([128, 2], F32)
    # ---- big persistent SBUF tensors -----------------------------------
    # host-transposed inputs, chunk-major for contiguous 4KB/partition DMA:
    # [p, lc, c, i] = X[512*lc + i, 128c+p]
    xqT = bigs.tile([128, 4, DC, 512], F16, tag="xqT")
    xkT = bigs.tile([128, 4, DC, 512], F16, tag="xkT")
    xvT = bigs.tile([128, 4, DC, 512], F16, tag="xvT")
    qT = bigs.tile([128, 2, L], F16, tag="qT")     # [e_in_chunk, ec, l]
    kT = bigs.tile([128, 2, S], F16, tag="kT")
    v_sb = bigs.tile([128, ST, HG, 65], F16, tag="v")  # [s_in_tile, st, h, dv+1]
    attnT = bigs.tile([128, 2, L], F16, tag="attnT")   # [64hh+e, pair, l]
    nc.vector.memset(v_sb[:, :, :, 64:65], 1.0)  # ones col -> Z row

    # input DMAs in need-order so compute can start early; the late-needed
    # wo and q quarters (lq 1-3) go last.
    def dma_in(dst, src, lc):
        nc.sync.dma_start(out=dst[:, lc], in_=src[:, lc])

    # k/v path issues on the SP ring; q path in parallel on the ACT ring
    # (idle until the first exp).  The first k/q chunks are split per
    # d-chunk so the first projection matmuls start ~4us earlier.
    # Issue stalls appear after ~8 outstanding DMAs per ring (sem lanes), so
    # split the later chunks across BOTH rings to land before their filler
    # deadlines instead of batching up behind one ring.
    for c in range(DC):
        nc.sync.dma_start(out=wk_sb[:, c], in_=io["wk"][:, c])
        nc.sync.dma_start(out=xkT[:, 0, c], in_=io["xkT"][:, 0, c])
    for c in range(DC):
        nc.scalar.dma_start(out=wq_sb[:, c], in_=io["wq"][:, c])
        nc.scalar.dma_start(out=xqT[:, 0, c], in_=io["xqT"][:, 0, c])
    nc.sync.dma_start(out=wv_sb, in_=io["wv"][:])
    dma_in(xvT, io["xvT"], 0)
    nc.scalar.dma_start(out=bk_sb, in_=io["bk"][:])
    nc.scalar.dma_start(out=bq_sb, in_=io["bq"][:])
    dma_in(xkT, io["xkT"], 1)
    dma_in(xvT, io["xvT"], 1)
    nc.scalar.dma_start(out=xkT[:, 2], in_=io["xkT"][:, 2])
    nc.scalar.dma_start(out=xvT[:, 2], in_=io["xvT"][:, 2])
    dma_in(xkT, io["xkT"], 3)
    dma_in(xvT, io["xvT"], 3)
    nc.scalar.dma_start(out=wo_sb, in_=io["wo"][:])
    for lc in range(1, 4):
        nc.scalar.dma_start(out=xqT[:, lc], in_=io["xqT"][:, lc])

    # ---- projections ----------------------------------------------------
    def proj_qk_ec(xT, w_sb, b_sb, dst, lc, ec):
        # dst[:, ec, 512lc : 512lc+512] = (W.T @ X^T) + bias, one e-chunk
        pp = ps_wk.tile([128, 512], F32, tag="work", name=f"pp_{lc}_{ec}_{dst.name}")
        for c in range(DC):
            mm(pp, lhsT=w_sb[:, c, ec * 128:(ec + 1) * 128],
               rhs=xT[:, lc, c, :],
               start=(c == 0), stop=(c == DC - 1))
        nc.vector.tensor_scalar_add(
            out=dst[:, ec, lc * 512:(lc + 1) * 512], in0=pp,
            scalar1=b_sb[:, ec:ec + 1])

    def proj_v_st(st):
        lc, i = st // 4, st % 4
        vp = ps_wk.tile([128, 512], F32, tag="work", name=f"vp_{st}")[:, 0:EG]
        for c in range(DC):
            mm(vp, lhsT=xvT[:, lc, c, i * 128:(i + 1) * 128],
               rhs=wv_sb[:, c, :], start=(c == 0), stop=(c == DC - 1))
        nc.vector.tensor_copy(
            out=v_sb[:, st, :, 0:64],
            in_=vp.rearrange("p (h e) -> p h e", h=HG))

    def oproj_lt(lq, lt, pool, tag):
        # out[l, 512] partial for one l-tile (attnT(lq) ready)
        op = pool.tile([128, D], F32, tag=tag, name=f"op_{lq}_{lt}")
        for p in range(2):
            mm(op, lhsT=attnT[:, p, lt * 128:(lt + 1) * 128],
               rhs=wo_sb[:, p, :], start=(p == 0), stop=(p == 1))
        ob = ob_pool.tile([128, D], F16, tag="ob")
        nc.vector.tensor_copy(out=ob, in_=op)
        nc.sync.dma_start(out=io["out"][lt * 128:(lt + 1) * 128, :], in_=ob)

    # prologue: only what the first attention tiles (p=0, j<4) need
    proj_qk_ec(xkT, wk_sb, bk_sb, kT, 0, 0)
    proj_qk_ec(xqT, wq_sb, bq_sb, qT, 0, 0)
    for st in range(4):
        proj_v_st(st)

    # ---- attention: one continuous scores->exp->AV stream ----------------
    # Filler units (~4 matmuls each) are emitted one-per-iteration AFTER the
    # iteration's attention ops, so the out-of-order scheduler slots them
    # into exp-wait gaps without starving the ACT engine with long bursts.
    def fillers_for(lq, p):
        fl = []
        if lq == 0 and p == 0:
            # remaining k/q e-chunks and v s-tiles, deadline-ordered
            # (scores j=4*lc need kT lc/ec0; AV j=st needs v st; the p=1
            # stream needs kT/qT ec1 from its j=0).
            def k0(lc, ec):
                return lambda: proj_qk_ec(xkT, wk_sb, bk_sb, kT, lc, ec)
            vs = [lambda st=st: proj_v_st(st) for st in range(4, 16)]
            q0e1 = lambda: proj_qk_ec(xqT, wq_sb, bq_sb, qT, 0, 1)
            return [[vs[0], k0(1, 0)], [vs[1], k0(0, 1)], [vs[2], vs[3]],
                    [q0e1], [vs[4], k0(2, 0)], [vs[5]], [vs[6], k0(3, 0)],
                    [vs[7]], [vs[8]], [vs[9]], [vs[10]], [vs[11]]]
        if lq == 0 and p == 1:
            # k e-chunk 1 for s-chunks 1-3 (read by this stream's j>=4)
            return [[lambda lc=lc: proj_qk_ec(xkT, wk_sb, bk_sb, kT, lc, 1)]
                    for lc in range(1, 4)] + \
                   [[lambda ec=ec: proj_qk_ec(xqT, wq_sb, bq_sb, qT, 1, ec)]
                    for ec in range(2)]
        out_l = []
        if p == 0:
            out_l += [[lambda lt=lt: oproj_lt(lq - 1, lt, ps_wk, "work")]
                      for lt in range((lq - 1) * 4, lq * 4)]
        elif lq + 1 < LQ:
            out_l += [[lambda ec=ec: proj_qk_ec(xqT, wq_sb, bq_sb, qT, lq + 1, ec)]
                      for ec in range(2)]
        return out_l

    for lq in range(LQ):
        l0 = lq * 512
        for p in range(2):                      # head pair (e-chunk)
            fill = fillers_for(lq, p)
            av = [ps_av.tile([65, 512], F32, tag="av", name=f"av{lq}_{p}_{i}")
                  for i in range(2)]
            for j in range(ST):
                sc = ps.tile([128, 2, 512], F32, tag="sc", name=f"sc_{lq}_{p}_{j}")
                ep = e_pool.tile([128, 2, 512], F16, tag="ep")
                for hh in range(2):             # rows 0-63 / 64-127: concurrent
                    o = hh * 64
                    mm(sc[:, hh, :],
                       lhsT=kT[o:o + 64, p, j * 128:(j + 1) * 128],
                       rhs=qT[o:o + 64, p, l0:l0 + 512],
                       start=True, stop=True, tile_position=(o, 0))
                nc.scalar.activation(out=ep, in_=sc,
                                     func=mybir.ActivationFunctionType.Exp)
                for hh in range(2):
                    mm(av[hh], lhsT=v_sb[:, j, 2 * p + hh, :],
                       rhs=ep[:, hh, :], start=(j == 0), stop=(j == ST - 1))
                if j < len(fill):
                    for f in fill[j]:
                        f()

            for grp in fill[ST:]:
                for f in grp:
                    f()

            # Z-normalize this head pair into attnT[:, p, l-quarter].
            # Copy the av accumulators out to SBUF immediately (releases the
            # PSUM slots so the next stream's AV never stalls), then run the
            # whole 1/Z chain off the critical path: one approx-recip on the
            # staged Z rows (two approx ops close together NaN on HW), GpSimd
            # broadcast, SBUF-side normalize muls.
            last = (lq == LQ - 1 and p == 1)
            zrow = z_pool.tile([1, 1024], F32, tag="zrow")
            if not last:
                avs = [z_pool.tile([64, 512], F32, tag=f"avs{i}",
                                   name=f"avs{lq}{p}{i}") for i in range(2)]
                nc.vector.tensor_copy(out=avs[0], in_=av[0][0:64, :])
                nc.vector.tensor_copy(out=zrow[0:1, 0:512], in_=av[0][64:65, :])
                nc.vector.tensor_copy(out=zrow[0:1, 512:1024],
                                      in_=av[1][64:65, :])
                nc.vector.tensor_copy(out=avs[1], in_=av[1][0:64, :])
            else:
                # tail: nothing queues behind the av slots; skip the staging
                # copies so the output projection starts sooner
                nc.vector.tensor_copy(out=zrow[0:1, 0:512], in_=av[0][64:65, :])
                nc.vector.tensor_copy(out=zrow[0:1, 512:1024],
                                      in_=av[1][64:65, :])
                avs = [av[0][0:64, :], av[1][0:64, :]]
            rrow = z_pool.tile([1, 1024], F32, tag="rrow")
            nc.vector.reciprocal_approx_fast(rrow, zrow)
            zbb = [z_pool.tile([64, 512], F32, tag="zbb", name=f"zbb{lq}{p}{i}")
                   for i in range(2)]
            for hh in range(2):
                nc.gpsimd.partition_broadcast(
                    zbb[hh], rrow[0:1, hh * 512:(hh + 1) * 512])
            # even head -> partitions 0:64, odd head -> 64:128 (quadrant-
            # routed DVE write; K=128 output projection reads the pair).
            nc.vector.tensor_mul(out=attnT[0:64, p, l0:l0 + 512],
                                 in0=avs[0], in1=zbb[0])
            nc.vector.tensor_mul(out=attnT[64:128, p, l0:l0 + 512],
                                 in0=avs[1], in1=zbb[1])

    # tail: output projection for the last l-quarter (av pool is free now)
    for lt in range((LQ - 1) * 4, LQ * 4):
        oproj_lt(LQ - 1, lt, ps_av, "av")


def build_nc():
    nc = bacc.Bacc()
    io = {}
    io["xqT"] = nc.declare_dram_parameter("xqT", [128, 4, DC, 512], F16, isOutput=False)
    io["xkT"] = nc.declare_dram_parameter("xkT", [128, 4, DC, 512], F16, isOutput=False)
    io["xvT"] = nc.declare_dram_parameter("xvT", [128, 4, DC, 512], F16, isOutput=False)
    io["wq"] = nc.declare_dram_parameter("wq", [128, DC, EG], F16, isOutput=False)
    io["wk"] = nc.declare_dram_parameter("wk", [128, DC, EG], F16, isOutput=False)
    io["wv"] = nc.declare_dram_parameter("wv", [128, DC, EG], F16, isOutput=False)
    io["wo"] = nc.declare_dram_parameter("wo", [128, 2, D], F16, isOutput=False)
    io["bq"] = nc.declare_dram_parameter("bq", [128, 2], F32, isOutput=False)
    io["bk"] = nc.declare_dram_parameter("bk", [128, 2], F32, isOutput=False)
    io["out"] = nc.declare_dram_parameter("out", [L, D], F16, isOutput=True)
    with tile.TileContext(nc) as tc:
        with ExitStack() as ctx:
            _emit(ctx, tc, io)
    nc.compile()
    return nc


_NC = None


def _get_nc():
    global _NC
    if _NC is None:
        _NC = build_nc()
    return _NC


def _chunk_w(w):
    """[512, n] -> [128, 4, n] fp16:  [p, c, :] = w[128c+p, :]"""
    n = w.shape[1]
    return np.ascontiguousarray(
        w.reshape(DC, 128, n).transpose(1, 0, 2), dtype=np.float16)


def _xT(x):
    """[L, 512] fp32 -> [128, 4, 4, 512] fp16:  [p, lc, c, i] = x[512lc+i, 128c+p]"""
    return np.ascontiguousarray(
        x.T.reshape(DC, 128, 4, 512).transpose(1, 2, 0, 3), dtype=np.float16)


def make_in_maps(queries, keys, values, tau, Wq, bq, Wk, bk, Wv, bv, Wo):
    in_maps = []
    xt_cache = {}
    for b in range(B):
        xt_cache[b] = (_xT(queries[b]), _xT(keys[b]), _xT(values[b]))
    for c in range(N_CORES):
        b, g = c // 2, c % 2
        e0 = g * EG
        f = np.float32(SCALE * tau[b])
        wq = _chunk_w(Wq[:, e0:e0 + EG] * f)
        wk = _chunk_w(Wk[:, e0:e0 + EG])
        wv = _chunk_w(Wv[:, e0:e0 + EG])
        wo = np.ascontiguousarray(
            Wo[e0:e0 + EG, :].reshape(2, 128, D).transpose(1, 0, 2),
            dtype=np.float16)
        xq, xk, xv = xt_cache[b]
        in_maps.append({
            "xqT": xq, "xkT": xk, "xvT": xv,
            "wq": wq, "wk": wk, "wv": wv, "wo": wo,
            "bq": np.ascontiguousarray(
                (bq[e0:e0 + EG] * f).reshape(2, 128).T, dtype=np.float32),
            "bk": np.ascontiguousarray(
                bk[e0:e0 + EG].reshape(2, 128).T, dtype=np.float32),
        })
    return in_maps


def kernel(queries, keys, values, tau, delta, Wq, bq, Wk, bk, Wv, bv, Wo, bo,
           **_unused):
    queries = np.asarray(queries, dtype=np.float32)
    keys = np.asarray(keys, dtype=np.float32)
    values = np.asarray(values, dtype=np.float32)
    tau = np.asarray(tau, dtype=np.float32)
    Wq, bq = np.asarray(Wq, np.float32), np.asarray(bq, np.float32)
    Wk, bk = np.asarray(Wk, np.float32), np.asarray(bk, np.float32)
    Wv, bv = np.asarray(Wv, np.float32), np.asarray(bv, np.float32)
    Wo, bo = np.asarray(Wo, np.float32), np.asarray(bo, np.float32)

    nc = _get_nc()
    in_maps = make_in_maps(queries, keys, values, tau, Wq, bq, Wk, bk, Wv, bv, Wo)
    res = run_bass_kernel_spmd(nc, in_maps, list(range(N_CORES)))
    # attn rows sum to 1 -> +bv flows through Wo as a constant row; + bo.
    const_row = (bv @ Wo + bo).astype(np.float32)  # [512]
    out = np.empty((B, L, D), dtype=np.float32)
    for b in range(B):
        out[b] = res.results[2 * b]["out"].astype(np.float32) \
            + res.results[2 * b + 1]["out"].astype(np.float32) + const_row
    return out


if __name__ == "__main__":
    nc = build_nc()
    print("built OK")


# revision 33
# speedup vs baseline: 1.0061x; 1.0061x over previous
"""DSAttention layer for Trainium2, 8 NeuronCores.

Sharding: core c -> batch b = c//2, head-group g = c%2 (4 heads each,
e-columns 256g..256g+255 of the 512-wide head dim).  tau[b]/8 (softmax
temperature x 1/sqrt(E)) is folded into each core's Wq/bq slice on the
host; delta[b] broadcasts over the softmax axis and is shift-invariant,
so it drops out exactly.  Each core emits its head-group's partial
output projection [2048, 512] fp16; the host sums the pair per batch
and adds (bv @ Wo + bo) in fp32.

v3: ACT-engine (exp) is the pacer (~142us of exp work).  The kernel is
one continuous scores->exp->AV stream per (l-quarter, head-pair); all
projection work (k/v/q proj, output proj) is emitted interleaved into
the stream so the out-of-order Tile scheduler uses it as PE filler and
the ACT engine never starves.  Z-normalization without PE transposes:
DVE reciprocal directly on the PSUM Z rows ([1,512] costs the same as
[128,512] on DVE), then a K=1 ones matmul broadcasts 1/Z across 64
partitions.  attnT is stored head-pair-stacked [128, 2, L] (odd head
on partitions 64:127 via DVE quadrant-routed writes) so the output
projection runs K=128 full-height (2 matmuls per l-tile instead of 4).
Input DMAs are issued chunk-interleaved (k0 q0 v0 k1 v1 ... q1 q2 q3)
so the first scores start ~10us in.

Device dataflow per core (all matmul operands fp16, fp32 PSUM accum):
  xT[q|k|v] [128, 4, 2048] fp16 DMA'd directly (host pre-transposed)
  qT/kT [e 256, l 2048] = W^T @ X^T   (e on partitions)
  v     [s 2048, e 256] -> fp16 v_aug [s,65] per head (ones col -> Z)
  scoresT[s,l] = kT.T @ qT  per head, head pairs concurrent via
                 partition-offset row groups (K=64 at rows 0-63/64-127)
  E = exp(scoresT - 2) fp16  (one ACT instr per [128, 2x512] pair tile)
  attnT_aug[65,l] = v_aug.T @ E  (accumulate 16 s-chunks in PSUM;
                 row 64 = softmax denominator Z)
  normalize: 1/Z row (DVE recip, psum->sbuf) -> K=1 matmul broadcast
                 -> attnT[128, pair, l] fp16 in SBUF
  out[l,512] = sum_p attnT[:, p, lt].T @ Wo_pair[:, p, :]  (K=128 x2)
"""

import numpy as np
from contextlib import ExitStack

import concourse.bass as bass
import concourse.bacc as bacc
import concourse.mybir as mybir
import concourse.tile as tile
from concourse.bass_utils import run_bass_kernel_spmd

F32 = mybir.dt.float32
F16 = mybir.dt.float16

B, L, S, D = 4, 2048, 2048, 512
H, E = 8, 64          # full model heads / head dim
HG = 4                # heads per core (head-group)
EG = HG * E           # 256, e-columns per core
N_CORES = 8

LT = L // 128         # 16 l-tiles
ST = S // 128         # 16 s-tiles
DC = D // 128         # 4 d-chunks
LQ = 4                # l-quarters of 512
SCALE = 1.0 / np.sqrt(np.float32(E))
EXP_SHIFT = -2.0      # exp(x-2): cancels in softmax, guards fp16 overflow


def _emit(ctx: ExitStack, tc: "tile.TileContext", io: dict):
    nc = tc.nc
    mm = nc.tensor.matmul

    singles = ctx.enter_context(tc.tile_pool(name="singles", bufs=1))
    bigs = ctx.enter_context(tc.tile_pool(name="bigs", bufs=1))
    e_pool = ctx.enter_context(tc.tile_pool(name="eslab", bufs=8))
    z_pool = ctx.enter_context(tc.tile_pool(name="zrec", bufs=2))
    ob_pool = ctx.enter_context(tc.tile_pool(name="outsb", bufs=3))

    # PSUM, statically 8 banks: sc 2x2 + av 3x1 + work 1x1.
    ps = ctx.enter_context(tc.tile_pool(name="ps", bufs=2, space="PSUM"))
    ps_av = ctx.enter_context(tc.tile_pool(name="ps_av", bufs=3, space="PSUM"))
    ps_wk = ctx.enter_context(tc.tile_pool(name="ps_wk", bufs=1, space="PSUM"))

    # ---- constants & weights -------------------------------------------
    wq_sb = singles.tile([128, DC, EG], F16)   # [p, c, e] = Wq[c*128+p, e]
    wk_sb = singles.tile([128, DC, EG], F16)
    wv_sb = singles.tile([128, DC, EG], F16)
    wo_sb = singles.tile([128, 2, D], F16)     # [r, p, n] = Wo[128p+r, n]
    bq_sb = singles.tile([128, 2], F32)        # [p, ec] = bq[128ec+p]
    bk_sb = singles.tile([128, 2], F32)
    # ---- big persistent SBUF tensors -----------------------------------
    # host-transposed inputs, chunk-major for contiguous 4KB/partition DMA:
    # [p, lc, c, i] = X[512*lc + i, 128c+p]
    xqT = bigs.tile([128, 4, DC, 512], F16, tag="xqT")
    xkT = bigs.tile([128, 4, DC, 512], F16, tag="xkT")
    xvT = bigs.tile([128, 4, DC, 512], F16, tag="xvT")
    qT = bigs.tile([128, 2, L], F16, tag="qT")     # [e_in_chunk, ec, l]
    kT = bigs.tile([128, 2, S], F16, tag="kT")
    v_sb = bigs.tile([128, ST, HG, 65], F16, tag="v")  # [s_in_tile, st, h, dv+1]
    attnT = bigs.tile([128, 2, L], F16, tag="attnT")   # [64hh+e, pair, l]
    nc.vector.memset(v_sb[:, :, :, 64:65], 1.0)  # ones col -> Z row

    # input DMAs in need-order so compute can start early; the late-needed
    # wo and q quarters (lq 1-3) go last.
    def dma_in(dst, src, lc):
        nc.sync.dma_start(out=dst[:, lc], in_=src[:, lc])

    # k/v path issues on the SP ring; q path in parallel on the ACT ring
    # (idle until the first exp).  The first k/q chunks are split per
    # d-chunk so the first projection matmuls start ~4us earlier.
    for c in range(DC):
        nc.sync.dma_start(out=wk_sb[:, c], in_=io["wk"][:, c])
        nc.sync.dma_start(out=xkT[:, 0, c], in_=io["xkT"][:, 0, c])
    for c in range(DC):
        nc.scalar.dma_start(out=wq_sb[:, c], in_=io["wq"][:, c])
        nc.scalar.dma_start(out=xqT[:, 0, c], in_=io["xqT"][:, 0, c])
    nc.sync.dma_start(out=bk_sb, in_=io["bk"][:])
    nc.sync.dma_start(out=bq_sb, in_=io["bq"][:])
    nc.sync.dma_start(out=wv_sb, in_=io["wv"][:])
    dma_in(xvT, io["xvT"], 0)
    dma_in(xkT, io["xkT"], 1)
    dma_in(xvT, io["xvT"], 1)
    dma_in(xkT, io["xkT"], 2)
    dma_in(xvT, io["xvT"], 2)
    dma_in(xkT, io["xkT"], 3)
    dma_in(xvT, io["xvT"], 3)
    nc.scalar.dma_start(out=wo_sb, in_=io["wo"][:])
    for lc in range(1, 4):
        nc.scalar.dma_start(out=xqT[:, lc], in_=io["xqT"][:, lc])

    # ---- projections ----------------------------------------------------
    def proj_qk_ec(xT, w_sb, b_sb, dst, lc, ec):
        # dst[:, ec, 512lc : 512lc+512] = (W.T @ X^T) + bias, one e-chunk
        pp = ps_wk.tile([128, 512], F32, tag="work", name=f"pp_{lc}_{ec}_{dst.name}")
        for c in range(DC):
            mm(pp, lhsT=w_sb[:, c, ec * 128:(ec + 1) * 128],
               rhs=xT[:, lc, c, :],
               start=(c == 0), stop=(c == DC - 1))
        nc.vector.tensor_scalar_add(
            out=dst[:, ec, lc * 512:(lc + 1) * 512], in0=pp,
            scalar1=b_sb[:, ec:ec + 1])

    def proj_v_st(st):
        lc, i = st // 4, st % 4
        vp = ps_wk.tile([128, 512], F32, tag="work", name=f"vp_{st}")[:, 0:EG]
        for c in range(DC):
            mm(vp, lhsT=xvT[:, lc, c, i * 128:(i + 1) * 128],
               rhs=wv_sb[:, c, :], start=(c == 0), stop=(c == DC - 1))
        nc.vector.tensor_copy(
            out=v_sb[:, st, :, 0:64],
            in_=vp.rearrange("p (h e) -> p h e", h=HG))

    def oproj_lt(lq, lt, pool, tag):
        # out[l, 512] partial for one l-tile (attnT(lq) ready)
        op = pool.tile([128, D], F32, tag=tag, name=f"op_{lq}_{lt}")
        for p in range(2):
            mm(op, lhsT=attnT[:, p, lt * 128:(lt + 1) * 128],
               rhs=wo_sb[:, p, :], start=(p == 0), stop=(p == 1))
        ob = ob_pool.tile([128, D], F16, tag="ob")
        nc.vector.tensor_copy(out=ob, in_=op)
        nc.sync.dma_start(out=io["out"][lt * 128:(lt + 1) * 128, :], in_=ob)

    # prologue: only what the first attention tiles (p=0, j<4) need
    proj_qk_ec(xkT, wk_sb, bk_sb, kT, 0, 0)
    proj_qk_ec(xqT, wq_sb, bq_sb, qT, 0, 0)
    for st in range(4):
        proj_v_st(st)

    # ---- attention: one continuous scores->exp->AV stream ----------------
    # Filler units (~4 matmuls each) are emitted one-per-iteration AFTER the
    # iteration's attention ops, so the out-of-order scheduler slots them
    # into exp-wait gaps without starving the ACT engine with long bursts.
    def fillers_for(lq, p):
        fl = []
        if lq == 0 and p == 0:
            # remaining k/q e-chunks and v s-tiles, deadline-ordered
            # (scores j=4*lc need kT lc/ec0; AV j=st needs v st; the p=1
            # stream needs kT/qT ec1 from its j=0).
            def k0(lc, ec):
                return lambda: proj_qk_ec(xkT, wk_sb, bk_sb, kT, lc, ec)
            vs = [lambda st=st: proj_v_st(st) for st in range(4, 16)]
            q0e1 = lambda: proj_qk_ec(xqT, wq_sb, bq_sb, qT, 0, 1)
            return [[vs[0], k0(1, 0)], [vs[1], k0(0, 1)], [vs[2], vs[3]],
                    [q0e1], [vs[4], k0(2, 0)], [vs[5]], [vs[6], k0(3, 0)],
                    [vs[7]], [vs[8]], [vs[9]], [vs[10]], [vs[11]]]
        if lq == 0 and p == 1:
            # k e-chunk 1 for s-chunks 1-3 (read by this stream's j>=4)
            return [[lambda lc=lc: proj_qk_ec(xkT, wk_sb, bk_sb, kT, lc, 1)]
                    for lc in range(1, 4)] + \
                   [[lambda ec=ec: proj_qk_ec(xqT, wq_sb, bq_sb, qT, 1, ec)]
                    for ec in range(2)]
        out_l = []
        if p == 0:
            out_l += [[lambda lt=lt: oproj_lt(lq - 1, lt, ps_wk, "work")]
                      for lt in range((lq - 1) * 4, lq * 4)]
        elif lq + 1 < LQ:
            out_l += [[lambda ec=ec: proj_qk_ec(xqT, wq_sb, bq_sb, qT, lq + 1, ec)]
                      for ec in range(2)]
        return out_l

    for lq in range(LQ):
        l0 = lq * 512
        for p in range(2):                      # head pair (e-chunk)
            fill = fillers_for(lq, p)
            av = [ps_av.tile([65, 512], F32, tag="av", name=f"av{lq}_{p}_{i}")
                  for i in range(2)]
            for j in range(ST):
                sc = ps.tile([128, 2, 512], F32, tag="sc", name=f"sc_{lq}_{p}_{j}")
                ep = e_pool.tile([128, 2, 512], F16, tag="ep")
                for hh in range(2):             # rows 0-63 / 64-127: concurrent
                    o = hh * 64
                    mm(sc[:, hh, :],
                       lhsT=kT[o:o + 64, p, j * 128:(j + 1) * 128],
                       rhs=qT[o:o + 64, p, l0:l0 + 512],
                       start=True, stop=True, tile_position=(o, 0))
                nc.scalar.activation(out=ep, in_=sc,
                                     func=mybir.ActivationFunctionType.Exp)
                for hh in range(2):
                    mm(av[hh], lhsT=v_sb[:, j, 2 * p + hh, :],
                       rhs=ep[:, hh, :], start=(j == 0), stop=(j == ST - 1))
                if j < len(fill):
                    for f in fill[j]:
                        f()

            for grp in fill[ST:]:
                for f in grp:
                    f()

            # Z-normalize this head pair into attnT[:, p, l-quarter].
            # Copy the av accumulators out to SBUF immediately (releases the
            # PSUM slots so the next stream's AV never stalls), then run the
            # whole 1/Z chain off the critical path: one approx-recip on the
            # staged Z rows (two approx ops close together NaN on HW), GpSimd
            # broadcast, SBUF-side normalize muls.
            last = (lq == LQ - 1 and p == 1)
            zrow = z_pool.tile([1, 1024], F32, tag="zrow")
            if not last:
                avs = [z_pool.tile([64, 512], F32, tag=f"avs{i}",
                                   name=f"avs{lq}{p}{i}") for i in range(2)]
                nc.vector.tensor_copy(out=avs[0], in_=av[0][0:64, :])
                nc.vector.tensor_copy(out=zrow[0:1, 0:512], in_=av[0][64:65, :])
                nc.vector.tensor_copy(out=zrow[0:1, 512:1024],
                                      in_=av[1][64:65, :])
                nc.vector.tensor_copy(out=avs[1], in_=av[1][0:64, :])
            else:
                # tail: nothing queues behind the av slots; skip the staging
                # copies so the output projection starts sooner
                nc.vector.tensor_copy(out=zrow[0:1, 0:512], in_=av[0][64:65, :])
                nc.vector.tensor_copy(out=zrow[0:1, 512:1024],
                                      in_=av[1][64:65, :])
                avs = [av[0][0:64, :], av[1][0:64, :]]
            rrow = z_pool.tile([1, 1024], F32, tag="rrow")
            nc.vector.reciprocal_approx_fast(rrow, zrow)
            zbb = [z_pool.tile([64, 512], F32, tag="zbb", name=f"zbb{lq}{p}{i}")
                   for i in range(2)]
            for hh in range(2):
                nc.gpsimd.partition_broadcast(
                    zbb[hh], rrow[0:1, hh * 512:(hh + 1) * 512])
            # even head -> partitions 0:64, odd head -> 64:128 (quadrant-
            # routed DVE write; K=128 output projection reads the pair).
            nc.vector.tensor_mul(out=attnT[0:64, p, l0:l0 + 512],
                                 in0=avs[0], in1=zbb[0])
            nc.vector.tensor_mul(out=attnT[64:128, p, l0:l0 + 512],
                                 in0=avs[1], in1=zbb[1])

    # tail: output projection for the last l-quarter (av pool is free now)
    for lt in range((LQ - 1) * 4, LQ * 4):
        oproj_lt(LQ - 1, lt, ps_av, "av")


def build_nc():
    nc = bacc.Bacc()
    io = {}
    io["xqT"] = nc.declare_dram_parameter("xqT", [128, 4, DC, 512], F16, isOutput=False)
    io["xkT"] = nc.declare_dram_parameter("xkT", [128, 4, DC, 512], F16, isOutput=False)
    io["xvT"] = nc.declare_dram_parameter("xvT", [128, 4, DC, 512], F16, isOutput=False)
    io["wq"] = nc.declare_dram_parameter("wq", [128, DC, EG], F16, isOutput=False)
    io["wk"] = nc.declare_dram_parameter("wk", [128, DC, EG], F16, isOutput=False)
    io["wv"] = nc.declare_dram_parameter("wv", [128, DC, EG], F16, isOutput=False)
    io["wo"] = nc.declare_dram_parameter("wo", [128, 2, D], F16, isOutput=False)
    io["bq"] = nc.declare_dram_parameter("bq", [128, 2], F32, isOutput=False)
    io["bk"] = nc.declare_dram_parameter("bk", [128, 2], F32, isOutput=False)
    io["out"] = nc.declare_dram_parameter("out", [L, D], F16, isOutput=True)
    with tile.TileContext(nc) as tc:
        with ExitStack() as ctx:
            _emit(ctx, tc, io)
    nc.compile()
    return nc


_NC = None


def _get_nc():
    global _NC
    if _NC is None:
        _NC = build_nc()
    return _NC


def _chunk_w(w):
    """[512, n] -> [128, 4, n] fp16:  [p, c, :] = w[128c+p, :]"""
    n = w.shape[1]
    return np.ascontiguousarray(
        w.reshape(DC, 128, n).transpose(1, 0, 2), dtype=np.float16)


def _xT(x):
    """[L, 512] fp32 -> [128, 4, 4, 512] fp16:  [p, lc, c, i] = x[512lc+i, 128c+p]"""
    return np.ascontiguousarray(
        x.T.reshape(DC, 128, 4, 512).transpose(1, 2, 0, 3), dtype=np.float16)


def make_in_maps(queries, keys, values, tau, Wq, bq, Wk, bk, Wv, bv, Wo):
    in_maps = []
    xt_cache = {}
    for b in range(B):
        xt_cache[b] = (_xT(queries[b]), _xT(keys[b]), _xT(values[b]))
    for c in range(N_CORES):
        b, g = c // 2, c % 2
        e0 = g * EG
        f = np.float32(SCALE * tau[b])
        wq = _chunk_w(Wq[:, e0:e0 + EG] * f)
        wk = _chunk_w(Wk[:, e0:e0 + EG])
        wv = _chunk_w(Wv[:, e0:e0 + EG])
        wo = np.ascontiguousarray(
            Wo[e0:e0 + EG, :].reshape(2, 128, D).transpose(1, 0, 2),
            dtype=np.float16)
        xq, xk, xv = xt_cache[b]
        in_maps.append({
            "xqT": xq, "xkT": xk, "xvT": xv,
            "wq": wq, "wk": wk, "wv": wv, "wo": wo,
            "bq": np.ascontiguousarray(
                (bq[e0:e0 + EG] * f).reshape(2, 128).T, dtype=np.float32),
            "bk": np.ascontiguousarray(
                bk[e0:e0 + EG].reshape(2, 128).T, dtype=np.float32),
        })
    return in_maps


def kernel(queries, keys, values, tau, delta, Wq, bq, Wk, bk, Wv, bv, Wo, bo,
           **_unused):
    queries = np.asarray(queries, dtype=np.float32)
    keys = np.asarray(keys, dtype=np.float32)
    values = np.asarray(values, dtype=np.float32)
    tau = np.asarray(tau, dtype=np.float32)
    Wq, bq = np.asarray(Wq, np.float32), np.asarray(bq, np.float32)
    Wk, bk = np.asarray(Wk, np.float32), np.asarray(bk, np.float32)
    Wv, bv = np.asarray(Wv, np.float32), np.asarray(bv, np.float32)
    Wo, bo = np.asarray(Wo, np.float32), np.asarray(bo, np.float32)

    nc = _get_nc()
    in_maps = make_in_maps(queries, keys, values, tau, Wq, bq, Wk, bk, Wv, bv, Wo)
    res = run_bass_kernel_spmd(nc, in_maps, list(range(N_CORES)))
    # attn rows sum to 1 -> +bv flows through Wo as a constant row; + bo.
    const_row = (bv @ Wo + bo).astype(np.float32)  # [512]
    out = np.empty((B, L, D), dtype=np.float32)
    for b in range(B):
        out[b] = res.results[2 * b]["out"].astype(np.float32) \
            + res.results[2 * b + 1]["out"].astype(np.float32) + const_row
    return out


if __name__ == "__main__":
    nc = build_nc()
    print("built OK")


# revision 34
# speedup vs baseline: 1.0071x; 1.0010x over previous
"""DSAttention layer for Trainium2, 8 NeuronCores.

Sharding: core c -> batch b = c//2, head-group g = c%2 (4 heads each,
e-columns 256g..256g+255 of the 512-wide head dim).  tau[b]/8 (softmax
temperature x 1/sqrt(E)) is folded into each core's Wq/bq slice on the
host; delta[b] broadcasts over the softmax axis and is shift-invariant,
so it drops out exactly.  Each core emits its head-group's partial
output projection [2048, 512] fp16; the host sums the pair per batch
and adds (bv @ Wo + bo) in fp32.

v3: ACT-engine (exp) is the pacer (~142us of exp work).  The kernel is
one continuous scores->exp->AV stream per (l-quarter, head-pair); all
projection work (k/v/q proj, output proj) is emitted interleaved into
the stream so the out-of-order Tile scheduler uses it as PE filler and
the ACT engine never starves.  Z-normalization without PE transposes:
DVE reciprocal directly on the PSUM Z rows ([1,512] costs the same as
[128,512] on DVE), then a K=1 ones matmul broadcasts 1/Z across 64
partitions.  attnT is stored head-pair-stacked [128, 2, L] (odd head
on partitions 64:127 via DVE quadrant-routed writes) so the output
projection runs K=128 full-height (2 matmuls per l-tile instead of 4).
Input DMAs are issued chunk-interleaved (k0 q0 v0 k1 v1 ... q1 q2 q3)
so the first scores start ~10us in.

Device dataflow per core (all matmul operands fp16, fp32 PSUM accum):
  xT[q|k|v] [128, 4, 2048] fp16 DMA'd directly (host pre-transposed)
  qT/kT [e 256, l 2048] = W^T @ X^T   (e on partitions)
  v     [s 2048, e 256] -> fp16 v_aug [s,65] per head (ones col -> Z)
  scoresT[s,l] = kT.T @ qT  per head, head pairs concurrent via
                 partition-offset row groups (K=64 at rows 0-63/64-127)
  E = exp(scoresT - 2) fp16  (one ACT instr per [128, 2x512] pair tile)
  attnT_aug[65,l] = v_aug.T @ E  (accumulate 16 s-chunks in PSUM;
                 row 64 = softmax denominator Z)
  normalize: 1/Z row (DVE recip, psum->sbuf) -> K=1 matmul broadcast
                 -> attnT[128, pair, l] fp16 in SBUF
  out[l,512] = sum_p attnT[:, p, lt].T @ Wo_pair[:, p, :]  (K=128 x2)
"""

import numpy as np
from contextlib import ExitStack

import concourse.bass as bass
import concourse.bacc as bacc
import concourse.mybir as mybir
import concourse.tile as tile
from concourse.bass_utils import run_bass_kernel_spmd

F32 = mybir.dt.float32
F16 = mybir.dt.float16

B, L, S, D = 4, 2048, 2048, 512
H, E = 8, 64          # full model heads / head dim
HG = 4                # heads per core (head-group)
EG = HG * E           # 256, e-columns per core
N_CORES = 8

LT = L // 128         # 16 l-tiles
ST = S // 128         # 16 s-tiles
DC = D // 128         # 4 d-chunks
LQ = 4                # l-quarters of 512
SCALE = 1.0 / np.sqrt(np.float32(E))
EXP_SHIFT = -2.0      # exp(x-2): cancels in softmax, guards fp16 overflow


def _emit(ctx: ExitStack, tc: "tile.TileContext", io: dict):
    nc = tc.nc
    mm = nc.tensor.matmul

    singles = ctx.enter_context(tc.tile_pool(name="singles", bufs=1))
    bigs = ctx.enter_context(tc.tile_pool(name="bigs", bufs=1))
    e_pool = ctx.enter_context(tc.tile_pool(name="eslab", bufs=6))
    z_pool = ctx.enter_context(tc.tile_pool(name="zrec", bufs=2))
    ob_pool = ctx.enter_context(tc.tile_pool(name="outsb", bufs=3))

    # PSUM, statically 8 banks: sc 2x2 + av 3x1 + work 1x1.
    ps = ctx.enter_context(tc.tile_pool(name="ps", bufs=2, space="PSUM"))
    ps_av = ctx.enter_context(tc.tile_pool(name="ps_av", bufs=3, space="PSUM"))
    ps_wk = ctx.enter_context(tc.tile_pool(name="ps_wk", bufs=1, space="PSUM"))

    # ---- constants & weights -------------------------------------------
    shift_col = singles.tile([128, 1], F32)
    nc.vector.memset(shift_col, EXP_SHIFT)

    wq_sb = singles.tile([128, DC, EG], F16)   # [p, c, e] = Wq[c*128+p, e]
    wk_sb = singles.tile([128, DC, EG], F16)
    wv_sb = singles.tile([128, DC, EG], F16)
    wo_sb = singles.tile([128, 2, D], F16)     # [r, p, n] = Wo[128p+r, n]
    bq_sb = singles.tile([128, 2], F32)        # [p, ec] = bq[128ec+p]
    bk_sb = singles.tile([128, 2], F32)
    # ---- big persistent SBUF tensors -----------------------------------
    # host-transposed inputs, chunk-major for contiguous 4KB/partition DMA:
    # [p, lc, c, i] = X[512*lc + i, 128c+p]
    xqT = bigs.tile([128, 4, DC, 512], F16, tag="xqT")
    xkT = bigs.tile([128, 4, DC, 512], F16, tag="xkT")
    xvT = bigs.tile([128, 4, DC, 512], F16, tag="xvT")
    qT = bigs.tile([128, 2, L], F16, tag="qT")     # [e_in_chunk, ec, l]
    kT = bigs.tile([128, 2, S], F16, tag="kT")
    v_sb = bigs.tile([128, ST, HG, 65], F16, tag="v")  # [s_in_tile, st, h, dv+1]
    attnT = bigs.tile([128, 2, L], F16, tag="attnT")   # [64hh+e, pair, l]
    nc.vector.memset(v_sb[:, :, :, 64:65], 1.0)  # ones col -> Z row

    # input DMAs in need-order so compute can start early; the late-needed
    # wo and q quarters (lq 1-3) go last.
    def dma_in(dst, src, lc):
        nc.sync.dma_start(out=dst[:, lc], in_=src[:, lc])

    # k/v path issues on the SP ring; q path in parallel on the ACT ring
    # (idle until the first exp).  The first k/q chunks are split per
    # d-chunk so the first projection matmuls start ~4us earlier.
    for c in range(DC):
        nc.sync.dma_start(out=wk_sb[:, c], in_=io["wk"][:, c])
        nc.sync.dma_start(out=xkT[:, 0, c], in_=io["xkT"][:, 0, c])
    for c in range(DC):
        nc.scalar.dma_start(out=wq_sb[:, c], in_=io["wq"][:, c])
        nc.scalar.dma_start(out=xqT[:, 0, c], in_=io["xqT"][:, 0, c])
    nc.sync.dma_start(out=bk_sb, in_=io["bk"][:])
    nc.sync.dma_start(out=bq_sb, in_=io["bq"][:])
    nc.sync.dma_start(out=wv_sb, in_=io["wv"][:])
    dma_in(xvT, io["xvT"], 0)
    dma_in(xkT, io["xkT"], 1)
    dma_in(xvT, io["xvT"], 1)
    dma_in(xkT, io["xkT"], 2)
    dma_in(xvT, io["xvT"], 2)
    dma_in(xkT, io["xkT"], 3)
    dma_in(xvT, io["xvT"], 3)
    nc.scalar.dma_start(out=wo_sb, in_=io["wo"][:])
    for lc in range(1, 4):
        nc.scalar.dma_start(out=xqT[:, lc], in_=io["xqT"][:, lc])

    # ---- projections ----------------------------------------------------
    def proj_qk_ec(xT, w_sb, b_sb, dst, lc, ec):
        # dst[:, ec, 512lc : 512lc+512] = (W.T @ X^T) + bias, one e-chunk
        pp = ps_wk.tile([128, 512], F32, tag="work", name=f"pp_{lc}_{ec}_{dst.name}")
        for c in range(DC):
            mm(pp, lhsT=w_sb[:, c, ec * 128:(ec + 1) * 128],
               rhs=xT[:, lc, c, :],
               start=(c == 0), stop=(c == DC - 1))
        nc.vector.tensor_scalar_add(
            out=dst[:, ec, lc * 512:(lc + 1) * 512], in0=pp,
            scalar1=b_sb[:, ec:ec + 1])

    def proj_v_st(st):
        lc, i = st // 4, st % 4
        vp = ps_wk.tile([128, 512], F32, tag="work", name=f"vp_{st}")[:, 0:EG]
        for c in range(DC):
            mm(vp, lhsT=xvT[:, lc, c, i * 128:(i + 1) * 128],
               rhs=wv_sb[:, c, :], start=(c == 0), stop=(c == DC - 1))
        nc.vector.tensor_copy(
            out=v_sb[:, st, :, 0:64],
            in_=vp.rearrange("p (h e) -> p h e", h=HG))

    def oproj_lt(lq, lt, pool, tag):
        # out[l, 512] partial for one l-tile (attnT(lq) ready)
        op = pool.tile([128, D], F32, tag=tag, name=f"op_{lq}_{lt}")
        for p in range(2):
            mm(op, lhsT=attnT[:, p, lt * 128:(lt + 1) * 128],
               rhs=wo_sb[:, p, :], start=(p == 0), stop=(p == 1))
        ob = ob_pool.tile([128, D], F16, tag="ob")
        nc.vector.tensor_copy(out=ob, in_=op)
        nc.sync.dma_start(out=io["out"][lt * 128:(lt + 1) * 128, :], in_=ob)

    # prologue: only what the first attention tiles (p=0, j<4) need
    proj_qk_ec(xkT, wk_sb, bk_sb, kT, 0, 0)
    proj_qk_ec(xqT, wq_sb, bq_sb, qT, 0, 0)
    for st in range(4):
        proj_v_st(st)

    # ---- attention: one continuous scores->exp->AV stream ----------------
    # Filler units (~4 matmuls each) are emitted one-per-iteration AFTER the
    # iteration's attention ops, so the out-of-order scheduler slots them
    # into exp-wait gaps without starving the ACT engine with long bursts.
    def fillers_for(lq, p):
        fl = []
        if lq == 0 and p == 0:
            # remaining k/q e-chunks and v s-tiles, deadline-ordered
            # (scores j=4*lc need kT lc/ec0; AV j=st needs v st; the p=1
            # stream needs kT/qT ec1 from its j=0).
            def k0(lc, ec):
                return lambda: proj_qk_ec(xkT, wk_sb, bk_sb, kT, lc, ec)
            vs = [lambda st=st: proj_v_st(st) for st in range(4, 16)]
            q0e1 = lambda: proj_qk_ec(xqT, wq_sb, bq_sb, qT, 0, 1)
            return [[vs[0], k0(1, 0)], [vs[1], k0(0, 1)], [vs[2], vs[3]],
                    [q0e1], [vs[4], k0(2, 0)], [vs[5]], [vs[6], k0(3, 0)],
                    [vs[7]], [vs[8]], [vs[9]], [vs[10]], [vs[11]]]
        if lq == 0 and p == 1:
            # k e-chunk 1 for s-chunks 1-3 (read by this stream's j>=4)
            return [[lambda lc=lc: proj_qk_ec(xkT, wk_sb, bk_sb, kT, lc, 1)]
                    for lc in range(1, 4)] + \
                   [[lambda ec=ec: proj_qk_ec(xqT, wq_sb, bq_sb, qT, 1, ec)]
                    for ec in range(2)]
        out_l = []
        if p == 0:
            out_l += [[lambda lt=lt: oproj_lt(lq - 1, lt, ps_wk, "work")]
                      for lt in range((lq - 1) * 4, lq * 4)]
        elif lq + 1 < LQ:
            out_l += [[lambda ec=ec: proj_qk_ec(xqT, wq_sb, bq_sb, qT, lq + 1, ec)]
                      for ec in range(2)]
        return out_l

    for lq in range(LQ):
        l0 = lq * 512
        for p in range(2):                      # head pair (e-chunk)
            fill = fillers_for(lq, p)
            av = [ps_av.tile([65, 512], F32, tag="av", name=f"av{lq}_{p}_{i}")
                  for i in range(2)]
            for j in range(ST):
                sc = ps.tile([128, 2, 512], F32, tag="sc", name=f"sc_{lq}_{p}_{j}")
                ep = e_pool.tile([128, 2, 512], F16, tag="ep")
                for hh in range(2):             # rows 0-63 / 64-127: concurrent
                    o = hh * 64
                    mm(sc[:, hh, :],
                       lhsT=kT[o:o + 64, p, j * 128:(j + 1) * 128],
                       rhs=qT[o:o + 64, p, l0:l0 + 512],
                       start=True, stop=True, tile_position=(o, 0))
                nc.scalar.activation(out=ep, in_=sc,
                                     func=mybir.ActivationFunctionType.Exp,
                                     bias=shift_col[:, 0:1], scale=1.0)
                for hh in range(2):
                    mm(av[hh], lhsT=v_sb[:, j, 2 * p + hh, :],
                       rhs=ep[:, hh, :], start=(j == 0), stop=(j == ST - 1))
                if j < len(fill):
                    for f in fill[j]:
                        f()

            for grp in fill[ST:]:
                for f in grp:
                    f()

            # Z-normalize this head pair into attnT[:, p, l-quarter].
            # Copy the av accumulators out to SBUF immediately (releases the
            # PSUM slots so the next stream's AV never stalls), then run the
            # whole 1/Z chain off the critical path: one approx-recip on the
            # staged Z rows (two approx ops close together NaN on HW), GpSimd
            # broadcast, SBUF-side normalize muls.
            last = (lq == LQ - 1 and p == 1)
            zrow = z_pool.tile([1, 1024], F32, tag="zrow")
            if not last:
                avs = [z_pool.tile([64, 512], F32, tag=f"avs{i}",
                                   name=f"avs{lq}{p}{i}") for i in range(2)]
                nc.vector.tensor_copy(out=avs[0], in_=av[0][0:64, :])
                nc.vector.tensor_copy(out=zrow[0:1, 0:512], in_=av[0][64:65, :])
                nc.vector.tensor_copy(out=zrow[0:1, 512:1024],
                                      in_=av[1][64:65, :])
                nc.vector.tensor_copy(out=avs[1], in_=av[1][0:64, :])
            else:
                # tail: nothing queues behind the av slots; skip the staging
                # copies so the output projection starts sooner
                nc.vector.tensor_copy(out=zrow[0:1, 0:512], in_=av[0][64:65, :])
                nc.vector.tensor_copy(out=zrow[0:1, 512:1024],
                                      in_=av[1][64:65, :])
                avs = [av[0][0:64, :], av[1][0:64, :]]
            rrow = z_pool.tile([1, 1024], F32, tag="rrow")
            nc.vector.reciprocal_approx_fast(rrow, zrow)
            zbb = [z_pool.tile([64, 512], F32, tag="zbb", name=f"zbb{lq}{p}{i}")
                   for i in range(2)]
            for hh in range(2):
                nc.gpsimd.partition_broadcast(
                    zbb[hh], rrow[0:1, hh * 512:(hh + 1) * 512])
            # even head -> partitions 0:64, odd head -> 64:128 (quadrant-
            # routed DVE write; K=128 output projection reads the pair).
            nc.vector.tensor_mul(out=attnT[0:64, p, l0:l0 + 512],
                                 in0=avs[0], in1=zbb[0])
            nc.vector.tensor_mul(out=attnT[64:128, p, l0:l0 + 512],
                                 in0=avs[1], in1=zbb[1])

    # tail: output projection for the last l-quarter (av pool is free now)
    for lt in range((LQ - 1) * 4, LQ * 4):
        oproj_lt(LQ - 1, lt, ps_av, "av")


def build_nc():
    nc = bacc.Bacc()
    io = {}
    io["xqT"] = nc.declare_dram_parameter("xqT", [128, 4, DC, 512], F16, isOutput=False)
    io["xkT"] = nc.declare_dram_parameter("xkT", [128, 4, DC, 512], F16, isOutput=False)
    io["xvT"] = nc.declare_dram_parameter("xvT", [128, 4, DC, 512], F16, isOutput=False)
    io["wq"] = nc.declare_dram_parameter("wq", [128, DC, EG], F16, isOutput=False)
    io["wk"] = nc.declare_dram_parameter("wk", [128, DC, EG], F16, isOutput=False)
    io["wv"] = nc.declare_dram_parameter("wv", [128, DC, EG], F16, isOutput=False)
    io["wo"] = nc.declare_dram_parameter("wo", [128, 2, D], F16, isOutput=False)
    io["bq"] = nc.declare_dram_parameter("bq", [128, 2], F32, isOutput=False)
    io["bk"] = nc.declare_dram_parameter("bk", [128, 2], F32, isOutput=False)
    io["out"] = nc.declare_dram_parameter("out", [L, D], F16, isOutput=True)
    with tile.TileContext(nc) as tc:
        with ExitStack() as ctx:
            _emit(ctx, tc, io)
    nc.compile()
    return nc


_NC = None


def _get_nc():
    global _NC
    if _NC is None:
        _NC = build_nc()
    return _NC


def _chunk_w(w):
    """[512, n] -> [128, 4, n] fp16:  [p, c, :] = w[128c+p, :]"""
    n = w.shape[1]
    return np.ascontiguousarray(
        w.reshape(DC, 128, n).transpose(1, 0, 2), dtype=np.float16)


def _xT(x):
    """[L, 512] fp32 -> [128, 4, 4, 512] fp16:  [p, lc, c, i] = x[512lc+i, 128c+p]"""
    return np.ascontiguousarray(
        x.T.reshape(DC, 128, 4, 512).transpose(1, 2, 0, 3), dtype=np.float16)


def make_in_maps(queries, keys, values, tau, Wq, bq, Wk, bk, Wv, bv, Wo):
    in_maps = []
    xt_cache = {}
    for b in range(B):
        xt_cache[b] = (_xT(queries[b]), _xT(keys[b]), _xT(values[b]))
    for c in range(N_CORES):
        b, g = c // 2, c % 2
        e0 = g * EG
        f = np.float32(SCALE * tau[b])
        wq = _chunk_w(Wq[:, e0:e0 + EG] * f)
        wk = _chunk_w(Wk[:, e0:e0 + EG])
        wv = _chunk_w(Wv[:, e0:e0 + EG])
        wo = np.ascontiguousarray(
            Wo[e0:e0 + EG, :].reshape(2, 128, D).transpose(1, 0, 2),
            dtype=np.float16)
        xq, xk, xv = xt_cache[b]
        in_maps.append({
            "xqT": xq, "xkT": xk, "xvT": xv,
            "wq": wq, "wk": wk, "wv": wv, "wo": wo,
            "bq": np.ascontiguousarray(
                (bq[e0:e0 + EG] * f).reshape(2, 128).T, dtype=np.float32),
            "bk": np.ascontiguousarray(
                bk[e0:e0 + EG].reshape(2, 128).T, dtype=np.float32),
        })
    return in_maps


def kernel(queries, keys, values, tau, delta, Wq, bq, Wk, bk, Wv, bv, Wo, bo,
           **_unused):
    queries = np.asarray(queries, dtype=np.float32)
    keys = np.asarray(keys, dtype=np.float32)
    values = np.asarray(values, dtype=np.float32)
    tau = np.asarray(tau, dtype=np.float32)
    Wq, bq = np.asarray(Wq, np.float32), np.asarray(bq, np.float32)
    Wk, bk = np.asarray(Wk, np.float32), np.asarray(bk, np.float32)
    Wv, bv = np.asarray(Wv, np.float32), np.asarray(bv, np.float32)
    Wo, bo = np.asarray(Wo, np.float32), np.asarray(bo, np.float32)

    nc = _get_nc()
    in_maps = make_in_maps(queries, keys, values, tau, Wq, bq, Wk, bk, Wv, bv, Wo)
    res = run_bass_kernel_spmd(nc, in_maps, list(range(N_CORES)))
    # attn rows sum to 1 -> +bv flows through Wo as a constant row; + bo.
    const_row = (bv @ Wo + bo).astype(np.float32)  # [512]
    out = np.empty((B, L, D), dtype=np.float32)
    for b in range(B):
        out[b] = res.results[2 * b]["out"].astype(np.float32) \
            + res.results[2 * b + 1]["out"].astype(np.float32) + const_row
    return out


if __name__ == "__main__":
    nc = build_nc()
    print("built OK")


# revision 35
# speedup vs baseline: 1.0335x; 1.0261x over previous
"""DSAttention layer for Trainium2, 8 NeuronCores.

Sharding: core c -> batch b = c//2, head-group g = c%2 (4 heads each,
e-columns 256g..256g+255 of the 512-wide head dim).  tau[b]/8 (softmax
temperature x 1/sqrt(E)) is folded into each core's Wq/bq slice on the
host; delta[b] broadcasts over the softmax axis and is shift-invariant,
so it drops out exactly.  Each core emits its head-group's partial
output projection [2048, 512] fp16; the host sums the pair per batch
and adds (bv @ Wo + bo) in fp32.

v3: ACT-engine (exp) is the pacer (~142us of exp work).  The kernel is
one continuous scores->exp->AV stream per (l-quarter, head-pair); all
projection work (k/v/q proj, output proj) is emitted interleaved into
the stream so the out-of-order Tile scheduler uses it as PE filler and
the ACT engine never starves.  Z-normalization without PE transposes:
DVE reciprocal directly on the PSUM Z rows ([1,512] costs the same as
[128,512] on DVE), then a K=1 ones matmul broadcasts 1/Z across 64
partitions.  attnT is stored head-pair-stacked [128, 2, L] (odd head
on partitions 64:127 via DVE quadrant-routed writes) so the output
projection runs K=128 full-height (2 matmuls per l-tile instead of 4).
Input DMAs are issued chunk-interleaved (k0 q0 v0 k1 v1 ... q1 q2 q3)
so the first scores start ~10us in.

Device dataflow per core (all matmul operands fp16, fp32 PSUM accum):
  xT[q|k|v] [128, 4, 2048] fp16 DMA'd directly (host pre-transposed)
  qT/kT [e 256, l 2048] = W^T @ X^T   (e on partitions)
  v     [s 2048, e 256] -> fp16 v_aug [s,65] per head (ones col -> Z)
  scoresT[s,l] = kT.T @ qT  per head, head pairs concurrent via
                 partition-offset row groups (K=64 at rows 0-63/64-127)
  E = exp(scoresT - 2) fp16  (one ACT instr per [128, 2x512] pair tile)
  attnT_aug[65,l] = v_aug.T @ E  (accumulate 16 s-chunks in PSUM;
                 row 64 = softmax denominator Z)
  normalize: 1/Z row (DVE recip, psum->sbuf) -> K=1 matmul broadcast
                 -> attnT[128, pair, l] fp16 in SBUF
  out[l,512] = sum_p attnT[:, p, lt].T @ Wo_pair[:, p, :]  (K=128 x2)
"""

import numpy as np
from contextlib import ExitStack

import concourse.bass as bass
import concourse.bacc as bacc
import concourse.mybir as mybir
import concourse.tile as tile
from concourse.bass_utils import run_bass_kernel_spmd

F32 = mybir.dt.float32
F16 = mybir.dt.float16

B, L, S, D = 4, 2048, 2048, 512
H, E = 8, 64          # full model heads / head dim
HG = 4                # heads per core (head-group)
EG = HG * E           # 256, e-columns per core
N_CORES = 8

LT = L // 128         # 16 l-tiles
ST = S // 128         # 16 s-tiles
DC = D // 128         # 4 d-chunks
LQ = 4                # l-quarters of 512
SCALE = 1.0 / np.sqrt(np.float32(E))
EXP_SHIFT = -2.0      # exp(x-2): cancels in softmax, guards fp16 overflow


def _emit(ctx: ExitStack, tc: "tile.TileContext", io: dict):
    nc = tc.nc
    mm = nc.tensor.matmul

    singles = ctx.enter_context(tc.tile_pool(name="singles", bufs=1))
    bigs = ctx.enter_context(tc.tile_pool(name="bigs", bufs=1))
    e_pool = ctx.enter_context(tc.tile_pool(name="eslab", bufs=6))
    z_pool = ctx.enter_context(tc.tile_pool(name="zrec", bufs=2))
    ob_pool = ctx.enter_context(tc.tile_pool(name="outsb", bufs=3))

    # PSUM, statically 8 banks: sc 2x2 + av 3x1 + work 1x1.
    ps = ctx.enter_context(tc.tile_pool(name="ps", bufs=2, space="PSUM"))
    ps_av = ctx.enter_context(tc.tile_pool(name="ps_av", bufs=3, space="PSUM"))
    ps_wk = ctx.enter_context(tc.tile_pool(name="ps_wk", bufs=1, space="PSUM"))

    # ---- constants & weights -------------------------------------------
    shift_col = singles.tile([128, 1], F32)
    nc.vector.memset(shift_col, EXP_SHIFT)

    wq_sb = singles.tile([128, DC, EG], F16)   # [p, c, e] = Wq[c*128+p, e]
    wk_sb = singles.tile([128, DC, EG], F16)
    wv_sb = singles.tile([128, DC, EG], F16)
    wo_sb = singles.tile([128, 2, D], F16)     # [r, p, n] = Wo[128p+r, n]
    bq_sb = singles.tile([128, 2], F32)        # [p, ec] = bq[128ec+p]
    bk_sb = singles.tile([128, 2], F32)
    # ---- big persistent SBUF tensors -----------------------------------
    # host-transposed inputs, chunk-major for contiguous 4KB/partition DMA:
    # [p, lc, c, i] = X[512*lc + i, 128c+p]
    xqT = bigs.tile([128, 4, DC, 512], F16, tag="xqT")
    xkT = bigs.tile([128, 4, DC, 512], F16, tag="xkT")
    xvT = bigs.tile([128, 4, DC, 512], F16, tag="xvT")
    qT = bigs.tile([128, 2, L], F16, tag="qT")     # [e_in_chunk, ec, l]
    kT = bigs.tile([128, 2, S], F16, tag="kT")
    v_sb = bigs.tile([128, ST, HG, 65], F16, tag="v")  # [s_in_tile, st, h, dv+1]
    attnT = bigs.tile([128, 2, L], F16, tag="attnT")   # [64hh+e, pair, l]
    nc.vector.memset(v_sb[:, :, :, 64:65], 1.0)  # ones col -> Z row

    # input DMAs in need-order so compute can start early; the late-needed
    # wo and q quarters (lq 1-3) go last.
    def dma_in(dst, src, lc):
        nc.sync.dma_start(out=dst[:, lc], in_=src[:, lc])

    # k/v path issues on the SP ring; q path in parallel on the ACT ring
    # (idle until the first exp).  The first k/q chunks are split per
    # d-chunk so the first projection matmuls start ~4us earlier.
    for c in range(DC):
        nc.sync.dma_start(out=wk_sb[:, c], in_=io["wk"][:, c])
        nc.sync.dma_start(out=xkT[:, 0, c], in_=io["xkT"][:, 0, c])
    for c in range(DC):
        nc.scalar.dma_start(out=wq_sb[:, c], in_=io["wq"][:, c])
        nc.scalar.dma_start(out=xqT[:, 0, c], in_=io["xqT"][:, 0, c])
    nc.sync.dma_start(out=bk_sb, in_=io["bk"][:])
    nc.sync.dma_start(out=bq_sb, in_=io["bq"][:])
    nc.sync.dma_start(out=wv_sb, in_=io["wv"][:])
    dma_in(xvT, io["xvT"], 0)
    dma_in(xkT, io["xkT"], 1)
    dma_in(xvT, io["xvT"], 1)
    dma_in(xkT, io["xkT"], 2)
    dma_in(xvT, io["xvT"], 2)
    dma_in(xkT, io["xkT"], 3)
    dma_in(xvT, io["xvT"], 3)
    nc.scalar.dma_start(out=wo_sb, in_=io["wo"][:])
    for lc in range(1, 4):
        nc.scalar.dma_start(out=xqT[:, lc], in_=io["xqT"][:, lc])

    # ---- projections ----------------------------------------------------
    def proj_qk_ec(xT, w_sb, b_sb, dst, lc, ec):
        # dst[:, ec, 512lc : 512lc+512] = (W.T @ X^T) + bias, one e-chunk
        pp = ps_wk.tile([128, 512], F32, tag="work", name=f"pp_{lc}_{ec}_{dst.name}")
        for c in range(DC):
            mm(pp, lhsT=w_sb[:, c, ec * 128:(ec + 1) * 128],
               rhs=xT[:, lc, c, :],
               start=(c == 0), stop=(c == DC - 1))
        nc.vector.tensor_scalar_add(
            out=dst[:, ec, lc * 512:(lc + 1) * 512], in0=pp,
            scalar1=b_sb[:, ec:ec + 1])

    def proj_v_st(st):
        lc, i = st // 4, st % 4
        vp = ps_wk.tile([128, 512], F32, tag="work", name=f"vp_{st}")[:, 0:EG]
        for c in range(DC):
            mm(vp, lhsT=xvT[:, lc, c, i * 128:(i + 1) * 128],
               rhs=wv_sb[:, c, :], start=(c == 0), stop=(c == DC - 1))
        nc.vector.tensor_copy(
            out=v_sb[:, st, :, 0:64],
            in_=vp.rearrange("p (h e) -> p h e", h=HG))

    def oproj_lt(lq, lt, pool, tag):
        # out[l, 512] partial for one l-tile (attnT(lq) ready)
        op = pool.tile([128, D], F32, tag=tag, name=f"op_{lq}_{lt}")
        for p in range(2):
            mm(op, lhsT=attnT[:, p, lt * 128:(lt + 1) * 128],
               rhs=wo_sb[:, p, :], start=(p == 0), stop=(p == 1))
        ob = ob_pool.tile([128, D], F16, tag="ob")
        nc.vector.tensor_copy(out=ob, in_=op)
        nc.sync.dma_start(out=io["out"][lt * 128:(lt + 1) * 128, :], in_=ob)

    # prologue: only what the first attention tiles (p=0, j<4) need
    proj_qk_ec(xkT, wk_sb, bk_sb, kT, 0, 0)
    proj_qk_ec(xqT, wq_sb, bq_sb, qT, 0, 0)
    for st in range(4):
        proj_v_st(st)

    # ---- attention: one continuous scores->exp->AV stream ----------------
    # Filler units (~4 matmuls each) are emitted one-per-iteration AFTER the
    # iteration's attention ops, so the out-of-order scheduler slots them
    # into exp-wait gaps without starving the ACT engine with long bursts.
    def fillers_for(lq, p):
        fl = []
        # Deadline-LATEST scheduling: each unit is emitted at the last safe
        # iteration, so its scheduler priority sits just below the attention
        # ops that need it.  Early-arriving DMA then cannot trigger a
        # low-priority batch drain that starves the ACT engine; late DMA
        # degrades exactly as before (the dep-blocked attention op forces
        # the filler to run next).
        slots = [[] for _ in range(ST)]

        def put(i, fn):
            slots[i].append(fn)

        def kf(lc, ec):
            return lambda: proj_qk_ec(xkT, wk_sb, bk_sb, kT, lc, ec)

        def qf(lc, ec):
            return lambda: proj_qk_ec(xqT, wq_sb, bq_sb, qT, lc, ec)

        if lq == 0 and p == 0:
            # kT lc/ec0 needed by scores j=4*lc; v st needed by AV j=st;
            # kT/qT ec1 of lc0 needed by the p=1 stream's j=0.
            put(2, kf(1, 0))
            put(7, kf(2, 0))
            put(11, kf(3, 0))
            for st in range(4, 16):
                put(min(st - 1, 14), lambda st=st: proj_v_st(st))
            put(13, kf(0, 1))
            put(14, qf(0, 1))
        elif lq == 0 and p == 1:
            put(2, kf(1, 1))
            put(6, kf(2, 1))
            put(10, kf(3, 1))
            put(13, qf(1, 0))
            put(14, qf(1, 1))
        elif p == 0:
            for i, lt in enumerate(range((lq - 1) * 4, lq * 4)):
                put(3 * i + 3, lambda lt=lt: oproj_lt(lq - 1, lt, ps_wk, "work"))
        elif lq + 1 < LQ:
            put(10, qf(lq + 1, 0))
            put(13, qf(lq + 1, 1))
        return slots

    for lq in range(LQ):
        l0 = lq * 512
        for p in range(2):                      # head pair (e-chunk)
            fill = fillers_for(lq, p)
            av = [ps_av.tile([65, 512], F32, tag="av", name=f"av{lq}_{p}_{i}")
                  for i in range(2)]
            for j in range(ST):
                sc = ps.tile([128, 2, 512], F32, tag="sc", name=f"sc_{lq}_{p}_{j}")
                ep = e_pool.tile([128, 2, 512], F16, tag="ep")
                for hh in range(2):             # rows 0-63 / 64-127: concurrent
                    o = hh * 64
                    mm(sc[:, hh, :],
                       lhsT=kT[o:o + 64, p, j * 128:(j + 1) * 128],
                       rhs=qT[o:o + 64, p, l0:l0 + 512],
                       start=True, stop=True, tile_position=(o, 0))
                nc.scalar.activation(out=ep, in_=sc,
                                     func=mybir.ActivationFunctionType.Exp,
                                     bias=shift_col[:, 0:1], scale=1.0)
                for hh in range(2):
                    mm(av[hh], lhsT=v_sb[:, j, 2 * p + hh, :],
                       rhs=ep[:, hh, :], start=(j == 0), stop=(j == ST - 1))
                if j < len(fill):
                    for f in fill[j]:
                        f()

            for grp in fill[ST:]:
                for f in grp:
                    f()

            # Z-normalize this head pair into attnT[:, p, l-quarter].
            # Copy the av accumulators out to SBUF immediately (releases the
            # PSUM slots so the next stream's AV never stalls), then run the
            # whole 1/Z chain off the critical path: one approx-recip on the
            # staged Z rows (two approx ops close together NaN on HW), GpSimd
            # broadcast, SBUF-side normalize muls.
            last = (lq == LQ - 1 and p == 1)
            zrow = z_pool.tile([1, 1024], F32, tag="zrow")
            if not last:
                avs = [z_pool.tile([64, 512], F32, tag=f"avs{i}",
                                   name=f"avs{lq}{p}{i}") for i in range(2)]
                nc.vector.tensor_copy(out=avs[0], in_=av[0][0:64, :])
                nc.vector.tensor_copy(out=zrow[0:1, 0:512], in_=av[0][64:65, :])
                nc.vector.tensor_copy(out=zrow[0:1, 512:1024],
                                      in_=av[1][64:65, :])
                nc.vector.tensor_copy(out=avs[1], in_=av[1][0:64, :])
            else:
                # tail: nothing queues behind the av slots; skip the staging
                # copies so the output projection starts sooner
                nc.vector.tensor_copy(out=zrow[0:1, 0:512], in_=av[0][64:65, :])
                nc.vector.tensor_copy(out=zrow[0:1, 512:1024],
                                      in_=av[1][64:65, :])
                avs = [av[0][0:64, :], av[1][0:64, :]]
            rrow = z_pool.tile([1, 1024], F32, tag="rrow")
            nc.vector.reciprocal_approx_fast(rrow, zrow)
            zbb = [z_pool.tile([64, 512], F32, tag="zbb", name=f"zbb{lq}{p}{i}")
                   for i in range(2)]
            for hh in range(2):
                nc.gpsimd.partition_broadcast(
                    zbb[hh], rrow[0:1, hh * 512:(hh + 1) * 512])
            # even head -> partitions 0:64, odd head -> 64:128 (quadrant-
            # routed DVE write; K=128 output projection reads the pair).
            nc.vector.tensor_mul(out=attnT[0:64, p, l0:l0 + 512],
                                 in0=avs[0], in1=zbb[0])
            nc.vector.tensor_mul(out=attnT[64:128, p, l0:l0 + 512],
                                 in0=avs[1], in1=zbb[1])

    # tail: output projection for the last l-quarter (av pool is free now)
    for lt in range((LQ - 1) * 4, LQ * 4):
        oproj_lt(LQ - 1, lt, ps_av, "av")


def build_nc():
    nc = bacc.Bacc()
    io = {}
    io["xqT"] = nc.declare_dram_parameter("xqT", [128, 4, DC, 512], F16, isOutput=False)
    io["xkT"] = nc.declare_dram_parameter("xkT", [128, 4, DC, 512], F16, isOutput=False)
    io["xvT"] = nc.declare_dram_parameter("xvT", [128, 4, DC, 512], F16, isOutput=False)
    io["wq"] = nc.declare_dram_parameter("wq", [128, DC, EG], F16, isOutput=False)
    io["wk"] = nc.declare_dram_parameter("wk", [128, DC, EG], F16, isOutput=False)
    io["wv"] = nc.declare_dram_parameter("wv", [128, DC, EG], F16, isOutput=False)
    io["wo"] = nc.declare_dram_parameter("wo", [128, 2, D], F16, isOutput=False)
    io["bq"] = nc.declare_dram_parameter("bq", [128, 2], F32, isOutput=False)
    io["bk"] = nc.declare_dram_parameter("bk", [128, 2], F32, isOutput=False)
    io["out"] = nc.declare_dram_parameter("out", [L, D], F16, isOutput=True)
    with tile.TileContext(nc) as tc:
        with ExitStack() as ctx:
            _emit(ctx, tc, io)
    nc.compile()
    return nc


_NC = None


def _get_nc():
    global _NC
    if _NC is None:
        _NC = build_nc()
    return _NC


def _chunk_w(w):
    """[512, n] -> [128, 4, n] fp16:  [p, c, :] = w[128c+p, :]"""
    n = w.shape[1]
    return np.ascontiguousarray(
        w.reshape(DC, 128, n).transpose(1, 0, 2), dtype=np.float16)


def _xT(x):
    """[L, 512] fp32 -> [128, 4, 4, 512] fp16:  [p, lc, c, i] = x[512lc+i, 128c+p]"""
    return np.ascontiguousarray(
        x.T.reshape(DC, 128, 4, 512).transpose(1, 2, 0, 3), dtype=np.float16)


def make_in_maps(queries, keys, values, tau, Wq, bq, Wk, bk, Wv, bv, Wo):
    in_maps = []
    xt_cache = {}
    for b in range(B):
        xt_cache[b] = (_xT(queries[b]), _xT(keys[b]), _xT(values[b]))
    for c in range(N_CORES):
        b, g = c // 2, c % 2
        e0 = g * EG
        f = np.float32(SCALE * tau[b])
        wq = _chunk_w(Wq[:, e0:e0 + EG] * f)
        wk = _chunk_w(Wk[:, e0:e0 + EG])
        wv = _chunk_w(Wv[:, e0:e0 + EG])
        wo = np.ascontiguousarray(
            Wo[e0:e0 + EG, :].reshape(2, 128, D).transpose(1, 0, 2),
            dtype=np.float16)
        xq, xk, xv = xt_cache[b]
        in_maps.append({
            "xqT": xq, "xkT": xk, "xvT": xv,
            "wq": wq, "wk": wk, "wv": wv, "wo": wo,
            "bq": np.ascontiguousarray(
                (bq[e0:e0 + EG] * f).reshape(2, 128).T, dtype=np.float32),
            "bk": np.ascontiguousarray(
                bk[e0:e0 + EG].reshape(2, 128).T, dtype=np.float32),
        })
    return in_maps


def kernel(queries, keys, values, tau, delta, Wq, bq, Wk, bk, Wv, bv, Wo, bo,
           **_unused):
    queries = np.asarray(queries, dtype=np.float32)
    keys = np.asarray(keys, dtype=np.float32)
    values = np.asarray(values, dtype=np.float32)
    tau = np.asarray(tau, dtype=np.float32)
    Wq, bq = np.asarray(Wq, np.float32), np.asarray(bq, np.float32)
    Wk, bk = np.asarray(Wk, np.float32), np.asarray(bk, np.float32)
    Wv, bv = np.asarray(Wv, np.float32), np.asarray(bv, np.float32)
    Wo, bo = np.asarray(Wo, np.float32), np.asarray(bo, np.float32)

    nc = _get_nc()
    in_maps = make_in_maps(queries, keys, values, tau, Wq, bq, Wk, bk, Wv, bv, Wo)
    res = run_bass_kernel_spmd(nc, in_maps, list(range(N_CORES)))
    # attn rows sum to 1 -> +bv flows through Wo as a constant row; + bo.
    const_row = (bv @ Wo + bo).astype(np.float32)  # [512]
    out = np.empty((B, L, D), dtype=np.float32)
    for b in range(B):
        out[b] = res.results[2 * b]["out"].astype(np.float32) \
            + res.results[2 * b + 1]["out"].astype(np.float32) + const_row
    return out


if __name__ == "__main__":
    nc = build_nc()
    print("built OK")
